# revision 1
# baseline (speedup 1.0000x reference)
"""Trainium2 Bass kernel for nn_DeepSpeedMoeWithJitter (8-core SPMD).

Strategy (data-parallel, dense-everything MoE):
  - Batch sharded 8 ways (1024 tokens/core); activations kept feature-major
    ("T" = transposed [feature, token]) so weights load in native layout.
  - Dense: h0 = relu(W0^T xT + b0), h1 = relu(W1^T h0 + b1)  [fp32r matmuls]
  - Gating: logits = Wg^T h1 (plain fp32), AllGather token-major logits
    [8192, 6]; every core redundantly computes the global top-2 + capacity
    cumsum with triangular-matrix matmuls (exact fp32 integer counts).
  - Per-token combine weights w[t, e] folded into the expert GEMM inputs:
    y = sum_e We[e]^T (h1 * w_e) + be^T w  — experts computed densely for the
    core's own tokens, accumulated in PSUM (fp32r).
  - Final: z = y^T Wp + bp (token-major out directly), log_softmax on-chip.
Only collective: one 196 KB AllGather. Per-core-varying inputs: x shard and
own-token index vector (for an indirect-DMA gather of global keep flags).
"""
import sys
import numpy as np

sys.path.insert(0, "/opt/trn_rl_repo")

import concourse.bass as bass
import concourse.bacc as bacc
import concourse.mybir as mybir
import concourse.tile as tile
from concourse import bass_utils

# problem dims (hardcoded per contract)
B, C_IN, H, W = 8192, 1, 64, 64
IN_DIM = 4096
M = 2048
NCLS = 1000
E = 6
CAP = 2731
NCORE = 8
TPC = B // NCORE          # 1024 tokens per core
NT = TPC // 128           # 8 token tiles per core
GT = B // 128             # 64 global token tiles
EPS = float(np.finfo(np.float32).eps)
BIG = 1e30

f32 = mybir.dt.float32
f32r = mybir.dt.float32r
i32 = mybir.dt.int32
AF = mybir.ActivationFunctionType
OP = mybir.AluOpType
AX = mybir.AxisListType


def rne12(a):
    """Round fp32 array to fp32r (11-bit mantissa): RNE at bit 12."""
    u = np.ascontiguousarray(a, np.float32).view(np.uint32).astype(np.uint64)
    r = (u + 0x7FF + ((u >> 12) & 1)) & 0xFFFFF000
    return r.astype(np.uint32).view(np.float32)



def build():
    nc = bacc.Bacc("TRN2", target_bir_lowering=False, debug=False,
                   num_devices=NCORE)

    # ---- I/O -----------------------------------------------------------
    xT_d = nc.dram_tensor("xT", [IN_DIM, TPC], f32r, kind="ExternalInput")
    W0_d = nc.dram_tensor("W0", [IN_DIM, M], f32r, kind="ExternalInput")
    b0_d = nc.dram_tensor("b0", [M, 1], f32, kind="ExternalInput")
    W1_d = nc.dram_tensor("W1", [M, M], f32r, kind="ExternalInput")
    b1_d = nc.dram_tensor("b1", [M, 1], f32, kind="ExternalInput")
    Wg_d = nc.dram_tensor("Wg", [M, E], f32, kind="ExternalInput")
    We_d = nc.dram_tensor("We", [E, M, M], f32r, kind="ExternalInput")
    be_d = nc.dram_tensor("be", [E, M], f32r, kind="ExternalInput")
    Wp_d = nc.dram_tensor("Wp", [M, NCLS], f32r, kind="ExternalInput")
    bp_d = nc.dram_tensor("bp", [1, NCLS], f32r, kind="ExternalInput")
    tri_d = nc.dram_tensor("tri", [128, 128], f32, kind="ExternalInput")
    triS_d = nc.dram_tensor("triS", [64, 64], f32, kind="ExternalInput")
    oidx_d = nc.dram_tensor("oidx", [TPC, 1], i32, kind="ExternalInput")
    selm_d = nc.dram_tensor("selm", [E, E * 128], f32, kind="ExternalInput")
    out_d = nc.dram_tensor("out", [TPC, NCLS], f32, kind="ExternalOutput")

    with tile.TileContext(nc) as tc:
        import contextlib
        with contextlib.ExitStack() as ctx:
            P_const = ctx.enter_context(tc.tile_pool(name="const", bufs=1))
            P_str = ctx.enter_context(tc.tile_pool(name="stream", bufs=3))
            P_dram = ctx.enter_context(tc.tile_pool(name="dram", bufs=1, space="DRAM"))

            # ---- constants in SBUF ------------------------------------
            tri_t = P_const.tile([128, 128], f32, tag="tri")
            triS_t = P_const.tile([64, 64], f32, tag="triS")
            ones_col = P_const.tile([128, 1], f32, tag="onescol")
            ones6 = P_const.tile([128, 6], f32, tag="ones6")
            ones6b = P_const.tile([64, 6], f32, tag="ones6b")
            pad_t = P_const.tile([128, 128], f32, tag="padt")
            ones_row = P_const.tile([1, 128], f32, tag="onesrow")
            ones_row_r = P_const.tile([1, 128], f32r, tag="onesrowr")
            ident = P_const.tile([128, 128], f32, tag="ident")
            selm_t = P_const.tile([E, E * 128], f32, tag="selm")
            nc.sync.dma_start(selm_t[:], selm_d[:, :])
            nc.sync.dma_start(tri_t[:], tri_d[:, :])
            from concourse.masks import make_identity
            make_identity(nc, ident[:])
            nc.sync.dma_start(triS_t[:], triS_d[:, :])
            nc.vector.memset(ones_col[:], 1.0)
            nc.vector.memset(ones6[:], 1.0)
            nc.vector.memset(ones6b[:], 1.0)
            nc.vector.memset(pad_t[:], 0.0)
            nc.vector.memset(ones_row[:], 1.0)
            nc.vector.tensor_copy(ones_row_r[:], ones_row[:])
            b0_ts, b1_ts, wg_ts = [], [], []
            for nt in range(16):
                b0_ts.append(P_const.tile([128, 1], f32, tag=f"b0_{nt}", name=f"b0_{nt}"))
                nc.sync.dma_start(b0_ts[nt][:], b0_d[nt * 128:(nt + 1) * 128, 0:1])
                b1_ts.append(P_const.tile([128, 1], f32, tag=f"b1_{nt}", name=f"b1_{nt}"))
                nc.sync.dma_start(b1_ts[nt][:], b1_d[nt * 128:(nt + 1) * 128, 0:1])
                wg_ts.append(P_const.tile([128, E], f32, tag=f"wg_{nt}", name=f"wg_{nt}"))
                nc.sync.dma_start(wg_ts[nt][:], Wg_d[nt * 128:(nt + 1) * 128, :])

            # ---- dense layers (h1 pool lives until end of expert stage)
            P_h1_cm = tc.tile_pool(name="h1", bufs=1)
            P_h1 = P_h1_cm.__enter__()
            h1T = [P_h1.tile([128, TPC], f32, tag=f"h1_{nt}", name=f"h1_{nt}")
                   for nt in range(16)]
            P_own_cm = tc.tile_pool(name="own", bufs=1)
            P_own = P_own_cm.__enter__()
            lg_own = [P_own.tile([128, E], f32, tag=f"lgo_{tt}", name=f"lgo_{tt}")
                      for tt in range(NT)]
            w_T = P_own.tile([E, TPC], f32, tag="w_T")

            with tc.tile_pool(name="h0", bufs=1) as P_h0:
                h0T = [P_h0.tile([128, TPC], f32r, tag=f"h0_{nt}", name=f"h0_{nt}")
                       for nt in range(16)]
                # layer 1: K=4096 (32 kt), 16 nt in 4 groups, 2 token chunks
                with tc.tile_pool(name="ps1", bufs=8, space="PSUM") as PS, \
                     tc.tile_pool(name="dstr1", bufs=4) as P_ds:
                    for tch in range(2):
                        tsl = slice(tch * 512, (tch + 1) * 512)
                        for ng in range(2):
                            acc = [PS.tile([128, 512], f32, tag="acc", name="acc")
                                   for _ in range(8)]
                            for kt in range(32):
                                xt = P_ds.tile([128, 512], f32r, tag="xts")
                                nc.sync.dma_start(
                                    xt[:], xT_d[kt * 128:(kt + 1) * 128, tsl])
                                w0 = P_ds.tile([128, 1024], f32r, tag="w0s")
                                nc.sync.dma_start(
                                    w0[:], W0_d[kt * 128:(kt + 1) * 128,
                                                ng * 1024:(ng + 1) * 1024])
                                for j in range(8):
                                    nc.tensor.matmul(
                                        acc[j][:], w0[:, j * 128:(j + 1) * 128],
                                        xt[:], start=(kt == 0), stop=(kt == 31))
                            for j in range(8):
                                nc.scalar.activation(
                                    h0T[ng * 8 + j][:, tsl], acc[j][:],
                                    AF.Relu, bias=b0_ts[ng * 8 + j][:, 0:1])
                # layer 2: K=2048 (16 kt)
                with tc.tile_pool(name="ps2", bufs=8, space="PSUM") as PS, \
                     tc.tile_pool(name="dstr2", bufs=4) as P_ds:
                    for tch in range(2):
                        tsl = slice(tch * 512, (tch + 1) * 512)
                        for ng in range(2):
                            acc = [PS.tile([128, 512], f32, tag="acc", name="acc")
                                   for _ in range(8)]
                            for kt in range(16):
                                w1 = P_ds.tile([128, 1024], f32r, tag="w1s")
                                nc.sync.dma_start(
                                    w1[:], W1_d[kt * 128:(kt + 1) * 128,
                                                ng * 1024:(ng + 1) * 1024])
                                for j in range(8):
                                    nc.tensor.matmul(
                                        acc[j][:], w1[:, j * 128:(j + 1) * 128],
                                        h0T[kt][:, tsl],
                                        start=(kt == 0), stop=(kt == 15))
                            for j in range(8):
                                nc.scalar.activation(
                                    h1T[ng * 8 + j][:, tsl], acc[j][:],
                                    AF.Relu, bias=b1_ts[ng * 8 + j][:, 0:1])

            # ---- logits (plain fp32) + AllGather ----------------------
            ag_in = P_dram.tile([TPC, E], f32, tag="ag_in")
            ag_out = P_dram.tile([B, E], f32, tag="ag_out", addr_space="Shared")
            R_dram = P_dram.tile([B, 2], f32, tag="Rt")
            cs_dram = P_dram.tile([2, GT * E], f32, tag="cs")
            off_dram = P_dram.tile([2, GT * E], f32, tag="off")
            y_dram = P_dram.tile([M, TPC], f32r, tag="y_dram")

            P_route_cm = tc.tile_pool(name="route", bufs=1)
            P_route = P_route_cm.__enter__()
            P_keep_cm = tc.tile_pool(name="keep", bufs=GT)
            P_keep = P_keep_cm.__enter__()

            with tc.tile_pool(name="pslg", bufs=1, space="PSUM") as PSL:
                lg_ps = PSL.tile([E, TPC], f32, tag="lg")
                for kt in range(16):
                    for th in range(2):
                        nc.tensor.matmul(
                            lg_ps[:, th * 512:(th + 1) * 512], wg_ts[kt][:],
                            h1T[kt][:, th * 512:(th + 1) * 512],
                            start=(kt == 0), stop=(kt == 15))
                lgT = P_route.tile([E, TPC], f32, tag="lgT")
                nc.vector.tensor_copy(lgT[:], lg_ps[:])
            with tc.tile_pool(name="pslt", bufs=4, space="PSUM") as PSLT, \
                 tc.tile_pool(name="padlt", bufs=2) as P_pad:
                for tt in range(NT):
                    padin = P_pad.tile([128, 128], f32, tag="padin")
                    nc.vector.tensor_copy(padin[:], pad_t[:])
                    nc.vector.tensor_copy(padin[0:E, :],
                                          lgT[:, tt * 128:(tt + 1) * 128])
                    tp_ps = PSLT.tile([128, 128], f32, tag="tp")
                    nc.tensor.transpose(tp_ps[:], padin[:], ident[:])
                    nc.vector.tensor_copy(lg_own[tt][:], tp_ps[:, 0:E])
                    nc.sync.dma_start(ag_in[tt * 128:(tt + 1) * 128, :],
                                      lg_own[tt][:])
            nc.gpsimd.collective_compute(
                "AllGather", OP.bypass,
                replica_groups=[list(range(NCORE))],
                ins=[ag_in[:]], outs=[ag_out[:]])

            # ---- global routing: masks + per-tile colsums --------------
            m1_all, m2_all = [], []
            with tc.tile_pool(name="ps3", bufs=1, space="PSUM") as PS3:
                cs1_ps = PS3.tile([E, GT * E], f32, tag="cs1")
                cs2_ps = PS3.tile([E, GT * E], f32, tag="cs2")
                for i in range(GT):
                    lg = P_str.tile([128, E], f32, tag="lga")
                    nc.sync.dma_start(lg[:], ag_out[i * 128:(i + 1) * 128, :])
                    rmax = P_str.tile([128, 1], f32, tag="rmax")
                    nc.vector.tensor_reduce(rmax[:], lg[:], AX.X, OP.max)
                    m1 = P_keep.tile([128, E], f32, tag="m1")
                    nc.vector.tensor_scalar(m1[:], lg[:], rmax[:, 0:1], None,
                                            OP.is_equal)
                    m1_all.append(m1)
                    l2n = P_str.tile([128, E], f32, tag="l2n")
                    nc.vector.scalar_tensor_tensor(
                        l2n[:], m1[:], BIG, lg[:], OP.mult, OP.subtract)
                    rmin = P_str.tile([128, 1], f32, tag="rmin")
                    nc.vector.tensor_reduce(rmin[:], l2n[:], AX.X, OP.min)
                    m2 = P_keep.tile([128, E], f32, tag="m2")
                    nc.vector.tensor_scalar(m2[:], l2n[:], rmin[:, 0:1], None,
                                            OP.is_equal)
                    m2_all.append(m2)
                    nc.tensor.matmul(cs1_ps[0:E, i * E:(i + 1) * E],
                                     ones6[:], m1[:], start=True, stop=True)
                    nc.tensor.matmul(cs2_ps[0:E, i * E:(i + 1) * E],
                                     ones6[:], m2[:], start=True, stop=True)
                cs1_sb = P_route.tile([1, GT * E], f32, tag="cs1_sb")
                cs2_sb = P_route.tile([1, GT * E], f32, tag="cs2_sb")
                nc.vector.tensor_copy(cs1_sb[:], cs1_ps[0:1, :])
                nc.vector.tensor_copy(cs2_sb[:], cs2_ps[0:1, :])
            nc.sync.dma_start(cs_dram[0:1, :], cs1_sb[:])
            nc.sync.dma_start(cs_dram[1:2, :], cs2_sb[:])

            # exclusive scan over per-tile colsums + top1 totals
            off_flat1 = P_route.tile([1, GT * E], f32, tag="off_flat1")
            off_flat2 = P_route.tile([1, GT * E], f32, tag="off_flat2")
            tot1_bc = P_route.tile([128, E], f32, tag="tot1bc")
            with tc.tile_pool(name="ps4", bufs=1, space="PSUM") as PS4:
                for s in range(2):
                    colr = P_str.tile([64, E], f32, tag="colr")
                    nc.sync.dma_start(
                        colr[:], cs_dram[s:s + 1, :].rearrange(
                            "p (i e) -> (p i) e", e=E))
                    off_ps = PS4.tile([64, E], f32, tag=f"off{s}", name=f"off{s}")
                    nc.tensor.matmul(off_ps[:], triS_t[:], colr[:],
                                     start=True, stop=True)
                    off_sb = P_str.tile([64, E], f32, tag="off_sb")
                    nc.vector.tensor_copy(off_sb[:], off_ps[:])
                    nc.sync.dma_start(
                        off_dram[s:s + 1, :].rearrange("p (i e) -> (p i) e", e=E),
                        off_sb[:])
                    if s == 0:
                        tot_ps = PS4.tile([E, E], f32, tag="tot")
                        nc.tensor.matmul(tot_ps[:], ones6b[:],
                                         colr[:], start=True, stop=True)
                        tot_sb = P_route.tile([1, E], f32, tag="tot_sb")
                        nc.vector.tensor_copy(tot_sb[:], tot_ps[0:1, :])
                        bc_ps = PS4.tile([128, E], f32, tag="bc")
                        nc.tensor.matmul(bc_ps[:], ones_row[:], tot_sb[:],
                                         start=True, stop=True)
                        nc.vector.tensor_copy(tot1_bc[:], bc_ps[:])
            nc.sync.dma_start(off_flat1[:, :], off_dram[0:1, :])
            nc.sync.dma_start(off_flat2[:, :], off_dram[1:2, :])

            # inclusive cumsums + capacity keep flags -> R_dram [8192, 2]
            with tc.tile_pool(name="ps5", bufs=8, space="PSUM") as PS5:
                for i in range(GT):
                    c1 = PS5.tile([128, E], f32, tag="cum")
                    nc.tensor.matmul(c1[:], tri_t[:], m1_all[i][:],
                                     start=True, stop=False)
                    nc.tensor.matmul(c1[:], ones_row[:],
                                     off_flat1[0:1, i * E:(i + 1) * E],
                                     start=False, stop=True)
                    c2 = PS5.tile([128, E], f32, tag="cum")
                    nc.tensor.matmul(c2[:], tri_t[:], m2_all[i][:],
                                     start=True, stop=False)
                    nc.tensor.matmul(c2[:], ones_row[:],
                                     off_flat2[0:1, i * E:(i + 1) * E],
                                     start=False, stop=True)
                    Rt = P_str.tile([128, 2], f32, tag="Rt")
                    scr = P_str.tile([128, E], f32, tag="scr")
                    a1 = P_str.tile([128, 1], f32, tag="a1")
                    nc.vector.tensor_mul(scr[:], m1_all[i][:], c1[:])
                    nc.vector.tensor_reduce(a1[:], scr[:], AX.X, OP.add)
                    nc.vector.tensor_scalar(Rt[:, 0:1], a1[:], float(CAP),
                                            None, OP.is_le)
                    c2e = P_str.tile([128, E], f32, tag="c2e")
                    nc.vector.tensor_add(c2e[:], c2[:], tot1_bc[:])
                    a2 = P_str.tile([128, 1], f32, tag="a2")
                    nc.vector.tensor_mul(scr[:], m2_all[i][:], c2e[:])
                    nc.vector.tensor_reduce(a2[:], scr[:], AX.X, OP.add)
                    nc.vector.tensor_scalar(Rt[:, 1:2], a2[:], float(CAP),
                                            None, OP.is_le)
                    nc.sync.dma_start(R_dram[i * 128:(i + 1) * 128, :], Rt[:])

            P_keep_cm.__exit__(None, None, None)
            P_route_cm.__exit__(None, None, None)

            # ---- own-token gates & combine weights w -------------------
            for tt in range(NT):
                oix = P_str.tile([128, 1], i32, tag="oix")
                nc.sync.dma_start(oix[:], oidx_d[tt * 128:(tt + 1) * 128, 0:1])
                keep = P_str.tile([128, 2], f32, tag="keepg")
                nc.gpsimd.indirect_dma_start(
                    out=keep[:], out_offset=None, in_=R_dram[:, :],
                    in_offset=bass.IndirectOffsetOnAxis(ap=oix[:, 0:1], axis=0))
                lg = lg_own[tt]
                rmax = P_str.tile([128, 1], f32, tag="rmax")
                nc.vector.tensor_reduce(rmax[:], lg[:], AX.X, OP.max)
                m1 = P_str.tile([128, E], f32, tag="m1o")
                nc.vector.tensor_scalar(m1[:], lg[:], rmax[:, 0:1], None,
                                        OP.is_equal)
                l2n = P_str.tile([128, E], f32, tag="l2n")
                nc.vector.scalar_tensor_tensor(
                    l2n[:], m1[:], BIG, lg[:], OP.mult, OP.subtract)
                rmin = P_str.tile([128, 1], f32, tag="rmin")
                nc.vector.tensor_reduce(rmin[:], l2n[:], AX.X, OP.min)
                m2 = P_str.tile([128, E], f32, tag="m2o")
                nc.vector.tensor_scalar(m2[:], l2n[:], rmin[:, 0:1], None,
                                        OP.is_equal)
                nmax = P_str.tile([128, 1], f32, tag="nmax")
                nc.vector.tensor_reduce(nmax[:], lg[:], AX.X, OP.max,
                                        negate=True)
                gates = P_str.tile([128, E], f32, tag="gates")
                sume = P_str.tile([128, 1], f32, tag="sume")
                nc.scalar.activation(gates[:], lg[:], AF.Exp,
                                     bias=nmax[:, 0:1])
                nc.vector.tensor_reduce(sume[:], gates[:], AX.X, OP.add)
                rsum = P_str.tile([128, 1], f32, tag="rsum")
                nc.vector.reciprocal(rsum[:], sume[:])
                nc.vector.tensor_scalar(gates[:], gates[:], rsum[:, 0:1],
                                        None, OP.mult)
                scr = P_str.tile([128, E], f32, tag="scr2")
                g1 = P_str.tile([128, 1], f32, tag="g1")
                nc.vector.tensor_mul(scr[:], gates[:], m1[:])
                nc.vector.tensor_reduce(g1[:], scr[:], AX.X, OP.add)
                g2 = P_str.tile([128, 1], f32, tag="g2")
                nc.vector.tensor_mul(scr[:], gates[:], m2[:])
                nc.vector.tensor_reduce(g2[:], scr[:], AX.X, OP.add)
                nc.vector.tensor_mul(g1[:], g1[:], keep[:, 0:1])
                nc.vector.tensor_mul(g2[:], g2[:], keep[:, 1:2])
                den = P_str.tile([128, 1], f32, tag="den")
                nc.vector.tensor_add(den[:], g1[:], g2[:])
                nc.vector.tensor_scalar(den[:], den[:], EPS, None, OP.max)
                rden = P_str.tile([128, 1], f32, tag="rden")
                nc.vector.reciprocal(rden[:], den[:])
                nc.vector.tensor_scalar(g1[:], g1[:], rden[:, 0:1], None, OP.mult)
                nc.vector.tensor_scalar(g2[:], g2[:], rden[:, 0:1], None, OP.mult)
                wt = P_str.tile([128, E], f32, tag="wt")
                nc.vector.tensor_scalar(wt[:], m1[:], g1[:, 0:1], None, OP.mult)
                w2 = P_str.tile([128, E], f32, tag="w2")
                nc.vector.tensor_scalar(w2[:], m2[:], g2[:, 0:1], None, OP.mult)
                nc.vector.tensor_add(wt[:], wt[:], w2[:])
                with tc.tile_pool(name=f"pswt_{tt}", bufs=1, space="PSUM") as PSW, \
                     tc.tile_pool(name=f"padwt_{tt}", bufs=1) as P_pw:
                    padw = P_pw.tile([128, 128], f32, tag="padw")
                    nc.vector.tensor_copy(padw[:], pad_t[:])
                    nc.vector.tensor_copy(padw[:, 0:E], wt[:])
                    wtp = PSW.tile([128, 128], f32, tag="wtp")
                    nc.tensor.transpose(wtp[:], padw[:], ident[:])
                    nc.vector.tensor_copy(w_T[:, tt * 128:(tt + 1) * 128],
                                          wtp[0:E, :])

            # ---- expert stage ------------------------------------------
            P_exw_cm = tc.tile_pool(name="exw", bufs=1)
            P_exw = P_exw_cm.__enter__()
            be_t = P_exw.tile([E, M], f32r, tag="be")
            nc.sync.dma_start(be_t[:], be_d[:, :])
            w_T_r = P_exw.tile([E, TPC], f32r, tag="w_T_r")
            nc.vector.tensor_copy(w_T_r[:], w_T[:])
            w_flat = P_exw.tile([1, E * TPC], f32, tag="w_flat")
            nc.sync.dma_start(w_flat[:], w_T[:])   # 6 partition lines -> 1 row

            for tch in range(2):
                tsl = slice(tch * 512, (tch + 1) * 512)
                # broadcast w columns for this token chunk: [128, 512] per e
                wbc = [P_exw.tile([128, 512], f32, tag=f"wbc_{e}",
                                  name=f"wbc_{e}") for e in range(E)]
                with tc.tile_pool(name=f"ps6_{tch}", bufs=2, space="PSUM") as PS6:
                    for e in range(E):
                        wb_ps = PS6.tile([128, 512], f32, tag="wb")
                        nc.tensor.matmul(
                            wb_ps[:], ones_row[:],
                            w_flat[0:1, e * TPC + tch * 512:
                                   e * TPC + (tch + 1) * 512],
                            start=True, stop=True)
                        nc.vector.tensor_copy(wbc[e][:], wb_ps[:])
                with tc.tile_pool(name=f"ps7_{tch}", bufs=8, space="PSUM") as PS7, \
                     tc.tile_pool(name=f"estr_{tch}", bufs=2) as P_es:
                    for nh in range(2):
                        acc = [PS7.tile([128, 512], f32, tag="acc", name="acc")
                               for _ in range(8)]
                        for e in range(E):
                            for kt in range(16):
                                h1w = P_es.tile([128, 512], f32r, tag="h1w")
                                nc.vector.tensor_mul(h1w[:], h1T[kt][:, tsl],
                                                     wbc[e][:])
                                we = P_es.tile([128, 1024], f32r, tag="wes")
                                nc.sync.dma_start(
                                    we[:], We_d[e, kt * 128:(kt + 1) * 128,
                                                nh * 1024:(nh + 1) * 1024])
                                for j in range(8):
                                    nc.tensor.matmul(
                                        acc[j][:], we[:, j * 128:(j + 1) * 128],
                                        h1w[:], start=(e == 0 and kt == 0),
                                        stop=False)
                        for j in range(8):
                            nc.tensor.matmul(
                                acc[j][:],
                                be_t[:, (nh * 8 + j) * 128:(nh * 8 + j + 1) * 128],
                                w_T_r[:, tsl], start=False, stop=True)
                            yst = P_es.tile([128, 512], f32r, tag="yst")
                            nc.vector.tensor_copy(yst[:], acc[j][:])
                            nc.sync.dma_start(
                                y_dram[(nh * 8 + j) * 128:(nh * 8 + j + 1) * 128,
                                       tsl], yst[:])

            P_exw_cm.__exit__(None, None, None)
            P_own_cm.__exit__(None, None, None)
            P_h1_cm.__exit__(None, None, None)

            # ---- final projection + log_softmax ------------------------
            P_z = ctx.enter_context(tc.tile_pool(name="z", bufs=1))
            z_sb = [P_z.tile([128, NCLS], f32, tag=f"z_{tt}", name=f"z_{tt}")
                    for tt in range(NT)]
            bp_t = P_z.tile([1, NCLS], f32r, tag="bp")
            nc.sync.dma_start(bp_t[:], bp_d[:, :])
            with tc.tile_pool(name="ps8", bufs=8, space="PSUM") as PS8, \
                 tc.tile_pool(name="zstr", bufs=3) as P_zs:
                for cch in range(2):
                    c0 = cch * 512
                    wc = min(512, NCLS - c0)
                    acc = [PS8.tile([128, 512], f32, tag="acc", name="acc")
                           for _ in range(NT)]
                    for kt in range(16):
                        wp = P_zs.tile([128, 512], f32r, tag="wps")
                        nc.sync.dma_start(
                            wp[:, 0:wc], Wp_d[kt * 128:(kt + 1) * 128,
                                              c0:c0 + wc])
                        yt = P_zs.tile([128, TPC], f32r, tag="yts")
                        nc.sync.dma_start(yt[:], y_dram[kt * 128:(kt + 1) * 128, :])
                        for tt in range(NT):
                            nc.tensor.matmul(
                                acc[tt][:, 0:wc],
                                yt[:, tt * 128:(tt + 1) * 128],
                                wp[:, 0:wc], start=(kt == 0), stop=False)
                    for tt in range(NT):
                        nc.tensor.matmul(acc[tt][:, 0:wc], ones_row_r[:],
                                         bp_t[0:1, c0:c0 + wc],
                                         start=False, stop=True)
                        nc.vector.tensor_copy(z_sb[tt][:, c0:c0 + wc],
                                              acc[tt][:, 0:wc])

            P_sm = ctx.enter_context(tc.tile_pool(name="smstr", bufs=3))
            for tt in range(NT):
                nmax = P_sm.tile([128, 1], f32, tag="zmax")
                nc.vector.tensor_reduce(nmax[:], z_sb[tt][:], AX.X, OP.max,
                                        negate=True)
                ez = P_sm.tile([128, NCLS], f32, tag="ez")
                sume = P_sm.tile([128, 1], f32, tag="zsum")
                nc.scalar.activation(ez[:], z_sb[tt][:], AF.Exp,
                                     bias=nmax[:, 0:1])
                nc.vector.tensor_reduce(sume[:], ez[:], AX.X, OP.add)
                lns = P_sm.tile([128, 1], f32, tag="lns")
                nc.scalar.activation(lns[:], sume[:], AF.Ln)
                o_t = P_sm.tile([128, NCLS], f32, tag="o_t")
                nc.vector.tensor_scalar(o_t[:], z_sb[tt][:], nmax[:, 0:1],
                                        None, OP.add)
                nc.vector.tensor_scalar(o_t[:], o_t[:], lns[:, 0:1],
                                        None, OP.subtract)
                nc.sync.dma_start(out_d[tt * 128:(tt + 1) * 128, :], o_t[:])

    nc.compile()
    return nc


_CACHE = {}


def _get_nc():
    if "nc" not in _CACHE:
        _CACHE["nc"] = build()
    return _CACHE["nc"]


def prepare_in_maps(x, W0, b0, W1, b1, Wg, We, be, Wp, bp):
    X = np.ascontiguousarray(np.asarray(x, np.float32).reshape(B, IN_DIM))
    shared = dict(
        W0=rne12(W0), b0=np.asarray(b0, np.float32).reshape(M, 1),
        W1=rne12(W1), b1=np.asarray(b1, np.float32).reshape(M, 1),
        Wg=np.asarray(Wg, np.float32),
        We=rne12(We), be=rne12(np.asarray(be, np.float32)),
        Wp=rne12(Wp), bp=rne12(np.asarray(bp, np.float32).reshape(1, NCLS)),
        tri=np.triu(np.ones((128, 128), np.float32)),
        selm=np.repeat(np.eye(E, dtype=np.float32), 128, axis=1),
        triS=np.triu(np.ones((64, 64), np.float32), 1),
    )
    in_maps = []
    for c in range(NCORE):
        xs = X[c * TPC:(c + 1) * TPC]
        in_maps.append(dict(
            shared,
            xT=rne12(np.ascontiguousarray(xs.T)),
            oidx=(c * TPC + np.arange(TPC, dtype=np.int32)).reshape(TPC, 1),
        ))
    return in_maps


def run_cores(inputs, trace=False):
    nc = _get_nc()
    in_maps = prepare_in_maps(**inputs)
    res = bass_utils.run_bass_kernel_spmd(
        nc, in_maps, core_ids=list(range(NCORE)), trace=trace)
    out = np.concatenate([res.results[c]["out"] for c in range(NCORE)], axis=0)
    return out, res


def kernel(**inputs) -> np.ndarray:
    out, _ = run_cores(inputs, trace=False)
    return out



# revision 4
# speedup vs baseline: 1.2513x; 1.2513x over previous
"""Trainium2 Bass kernel for nn_DeepSpeedMoeWithJitter (8-core SPMD).

Strategy (data-parallel, dense-everything MoE):
  - Batch sharded 8 ways (1024 tokens/core); activations kept feature-major
    ("T" = transposed [feature, token]) so weights load in native layout.
  - Dense: h0 = relu(W0^T xT + b0), h1 = relu(W1^T h0 + b1)  [fp32r matmuls]
  - Gating: logits = Wg^T h1 (plain fp32), AllGather token-major logits
    [8192, 6]; every core redundantly computes the global top-2 + capacity
    cumsum with triangular-matrix matmuls (exact fp32 integer counts).
  - Per-token combine weights w[t, e] folded into the expert GEMM inputs:
    y = sum_e We[e]^T (h1 * w_e) + be^T w  — experts computed densely for the
    core's own tokens, accumulated in PSUM (fp32r).
  - Final: z = y^T Wp + bp (token-major out directly), log_softmax on-chip.
Only collective: one 196 KB AllGather. Per-core-varying inputs: x shard and
own-token index vector (for an indirect-DMA gather of global keep flags).
"""
import sys
import numpy as np

sys.path.insert(0, "/opt/trn_rl_repo")

import concourse.bass as bass
import concourse.bacc as bacc
import concourse.mybir as mybir
import concourse.tile as tile
from concourse import bass_utils

# problem dims (hardcoded per contract)
B, C_IN, H, W = 8192, 1, 64, 64
IN_DIM = 4096
M = 2048
NCLS = 1000
E = 6
CAP = 2731
NCORE = 8
TPC = B // NCORE          # 1024 tokens per core
NT = TPC // 128           # 8 token tiles per core
GT = B // 128             # 64 global token tiles
EPS = float(np.finfo(np.float32).eps)
BIG = 1e30

f32 = mybir.dt.float32
f32r = mybir.dt.float32r
i32 = mybir.dt.int32
AF = mybir.ActivationFunctionType
OP = mybir.AluOpType
AX = mybir.AxisListType


def rne12(a):
    """Round fp32 array to fp32r (11-bit mantissa): RNE at bit 12."""
    u = np.ascontiguousarray(a, np.float32).view(np.uint32).astype(np.uint64)
    r = (u + 0x7FF + ((u >> 12) & 1)) & 0xFFFFF000
    return r.astype(np.uint32).view(np.float32)



def build(single_core=False):
    nc = bacc.Bacc("TRN2", target_bir_lowering=False, debug=False,
                   num_devices=(1 if single_core else NCORE))

    # ---- I/O -----------------------------------------------------------
    xT_d = nc.dram_tensor("xT", [IN_DIM, TPC], f32r, kind="ExternalInput")
    W0_d = nc.dram_tensor("W0", [IN_DIM, M], f32r, kind="ExternalInput")
    b0_d = nc.dram_tensor("b0", [M, 1], f32, kind="ExternalInput")
    W1_d = nc.dram_tensor("W1", [M, M], f32r, kind="ExternalInput")
    b1_d = nc.dram_tensor("b1", [M, 1], f32, kind="ExternalInput")
    Wg_d = nc.dram_tensor("Wg", [M, E], f32, kind="ExternalInput")
    We_d = nc.dram_tensor("We", [E, M, M], f32r, kind="ExternalInput")
    be_d = nc.dram_tensor("be", [E, M], f32r, kind="ExternalInput")
    Wp_d = nc.dram_tensor("Wp", [M, NCLS], f32r, kind="ExternalInput")
    bp_d = nc.dram_tensor("bp", [1, NCLS], f32r, kind="ExternalInput")
    tri_d = nc.dram_tensor("tri", [128, 128], f32, kind="ExternalInput")
    triS_d = nc.dram_tensor("triS", [64, 64], f32, kind="ExternalInput")
    oidx_d = nc.dram_tensor("oidx", [TPC, 1], i32, kind="ExternalInput")
    selm_d = nc.dram_tensor("selm", [E, E * 128], f32, kind="ExternalInput")
    out_d = nc.dram_tensor("out", [TPC, NCLS], f32, kind="ExternalOutput")

    with tile.TileContext(nc) as tc:
        import contextlib
        with contextlib.ExitStack() as ctx:
            P_const = ctx.enter_context(tc.tile_pool(name="const", bufs=1))
            P_str = ctx.enter_context(tc.tile_pool(name="stream", bufs=3))
            P_dram = ctx.enter_context(tc.tile_pool(name="dram", bufs=1, space="DRAM"))

            # ---- constants in SBUF ------------------------------------
            tri_t = P_const.tile([128, 128], f32, tag="tri")
            triS_t = P_const.tile([64, 64], f32, tag="triS")
            ones_col = P_const.tile([128, 1], f32, tag="onescol")
            ones6 = P_const.tile([128, 6], f32, tag="ones6")
            ones6b = P_const.tile([64, 6], f32, tag="ones6b")
            pad_t = P_const.tile([128, 128], f32, tag="padt")
            ones_row = P_const.tile([1, 128], f32, tag="onesrow")
            ones_row_r = P_const.tile([1, 128], f32r, tag="onesrowr")
            ident = P_const.tile([128, 128], f32, tag="ident")
            selm_t = P_const.tile([E, E * 128], f32, tag="selm")
            nc.sync.dma_start(selm_t[:], selm_d[:, :])
            nc.sync.dma_start(tri_t[:], tri_d[:, :])
            from concourse.masks import make_identity
            make_identity(nc, ident[:])
            nc.sync.dma_start(triS_t[:], triS_d[:, :])
            nc.vector.memset(ones_col[:], 1.0)
            nc.vector.memset(ones6[:], 1.0)
            nc.vector.memset(ones6b[:], 1.0)
            nc.vector.memset(pad_t[:], 0.0)
            nc.vector.memset(ones_row[:], 1.0)
            nc.vector.tensor_copy(ones_row_r[:], ones_row[:])
            b0_ts, b1_ts, wg_ts = [], [], []
            for nt in range(16):
                b0_ts.append(P_const.tile([128, 1], f32, tag=f"b0_{nt}", name=f"b0_{nt}"))
                nc.sync.dma_start(b0_ts[nt][:], b0_d[nt * 128:(nt + 1) * 128, 0:1])
                b1_ts.append(P_const.tile([128, 1], f32, tag=f"b1_{nt}", name=f"b1_{nt}"))
                nc.sync.dma_start(b1_ts[nt][:], b1_d[nt * 128:(nt + 1) * 128, 0:1])
                wg_ts.append(P_const.tile([128, E], f32, tag=f"wg_{nt}", name=f"wg_{nt}"))
                nc.sync.dma_start(wg_ts[nt][:], Wg_d[nt * 128:(nt + 1) * 128, :])

            # ---- dense layers (h1 pool lives until end of expert stage)
            P_h1_cm = tc.tile_pool(name="h1", bufs=1)
            P_h1 = P_h1_cm.__enter__()
            h1T = [P_h1.tile([128, TPC], f32, tag=f"h1_{nt}", name=f"h1_{nt}")
                   for nt in range(16)]
            P_own_cm = tc.tile_pool(name="own", bufs=1)
            P_own = P_own_cm.__enter__()
            lg_own = [P_own.tile([128, E], f32, tag=f"lgo_{tt}", name=f"lgo_{tt}")
                      for tt in range(NT)]
            w_T = P_own.tile([E, TPC], f32, tag="w_T")

            with tc.tile_pool(name="h0", bufs=1) as P_h0:
                h0T = [P_h0.tile([128, TPC], f32r, tag=f"h0_{nt}", name=f"h0_{nt}")
                       for nt in range(16)]
                # layer 1: K=4096 (32 kt), 16 nt in 4 groups, 2 token chunks
                with tc.tile_pool(name="ps1", bufs=8, space="PSUM") as PS, \
                     tc.tile_pool(name="dstr1", bufs=4) as P_ds:
                    for tch in range(2):
                        tsl = slice(tch * 512, (tch + 1) * 512)
                        for ng in range(2):
                            acc = [PS.tile([128, 512], f32, tag="acc", name="acc")
                                   for _ in range(8)]
                            for kt in range(32):
                                xt = P_ds.tile([128, 512], f32r, tag="xts")
                                nc.sync.dma_start(
                                    xt[:], xT_d[kt * 128:(kt + 1) * 128, tsl])
                                w0 = P_ds.tile([128, 1024], f32r, tag="w0s")
                                nc.sync.dma_start(
                                    w0[:], W0_d[kt * 128:(kt + 1) * 128,
                                                ng * 1024:(ng + 1) * 1024])
                                for j in range(8):
                                    nc.tensor.matmul(
                                        acc[j][:], w0[:, j * 128:(j + 1) * 128],
                                        xt[:], start=(kt == 0), stop=(kt == 31))
                            for j in range(8):
                                nc.scalar.activation(
                                    h0T[ng * 8 + j][:, tsl], acc[j][:],
                                    AF.Relu, bias=b0_ts[ng * 8 + j][:, 0:1])
                # layer 2: K=2048 (16 kt)
                with tc.tile_pool(name="ps2", bufs=8, space="PSUM") as PS, \
                     tc.tile_pool(name="dstr2", bufs=4) as P_ds:
                    for tch in range(2):
                        tsl = slice(tch * 512, (tch + 1) * 512)
                        for ng in range(2):
                            acc = [PS.tile([128, 512], f32, tag="acc", name="acc")
                                   for _ in range(8)]
                            for kt in range(16):
                                w1 = P_ds.tile([128, 1024], f32r, tag="w1s")
                                nc.sync.dma_start(
                                    w1[:], W1_d[kt * 128:(kt + 1) * 128,
                                                ng * 1024:(ng + 1) * 1024])
                                for j in range(8):
                                    nc.tensor.matmul(
                                        acc[j][:], w1[:, j * 128:(j + 1) * 128],
                                        h0T[kt][:, tsl],
                                        start=(kt == 0), stop=(kt == 15))
                            for j in range(8):
                                nc.scalar.activation(
                                    h1T[ng * 8 + j][:, tsl], acc[j][:],
                                    AF.Relu, bias=b1_ts[ng * 8 + j][:, 0:1])

            # ---- logits (plain fp32) + AllGather ----------------------
            ag_in = P_dram.tile([TPC, E], f32, tag="ag_in")
            ag_out = P_dram.tile([B, E], f32, tag="ag_out",
                                 **({} if single_core
                                    else dict(addr_space="Shared")))
            R_dram = P_dram.tile([B, 2], f32, tag="Rt")
            cs_dram = P_dram.tile([2, GT * E], f32, tag="cs")
            off_dram = P_dram.tile([2, GT * E], f32, tag="off")
            y_dram = P_dram.tile([M, TPC], f32r, tag="y_dram")

            P_route_cm = tc.tile_pool(name="route", bufs=1)
            P_route = P_route_cm.__enter__()
            P_keep_cm = tc.tile_pool(name="keep", bufs=GT)
            P_keep = P_keep_cm.__enter__()

            with tc.tile_pool(name="pslg", bufs=1, space="PSUM") as PSL:
                lg_ps = PSL.tile([E, TPC], f32, tag="lg")
                for kt in range(16):
                    for th in range(2):
                        nc.tensor.matmul(
                            lg_ps[:, th * 512:(th + 1) * 512], wg_ts[kt][:],
                            h1T[kt][:, th * 512:(th + 1) * 512],
                            start=(kt == 0), stop=(kt == 15))
                lgT = P_route.tile([E, TPC], f32, tag="lgT")
                nc.vector.tensor_copy(lgT[:], lg_ps[:])
            with tc.tile_pool(name="pslt", bufs=4, space="PSUM") as PSLT, \
                 tc.tile_pool(name="padlt", bufs=2) as P_pad:
                for tt in range(NT):
                    padin = P_pad.tile([128, 128], f32, tag="padin")
                    nc.vector.tensor_copy(padin[:], pad_t[:])
                    nc.vector.tensor_copy(padin[0:E, :],
                                          lgT[:, tt * 128:(tt + 1) * 128])
                    tp_ps = PSLT.tile([128, 128], f32, tag="tp")
                    nc.tensor.transpose(tp_ps[:], padin[:], ident[:])
                    nc.vector.tensor_copy(lg_own[tt][:], tp_ps[:, 0:E])
                    nc.sync.dma_start(ag_in[tt * 128:(tt + 1) * 128, :],
                                      lg_own[tt][:])
            if single_core:
                # timing-analysis stand-in for the AllGather: same local
                # read/write volume via plain DRAM copies
                for r in range(NCORE):
                    nc.sync.dma_start(ag_out[r * TPC:(r + 1) * TPC, :],
                                      ag_in[:, :])
            else:
                nc.gpsimd.collective_compute(
                    "AllGather", OP.bypass,
                    replica_groups=[list(range(NCORE))],
                    ins=[ag_in[:]], outs=[ag_out[:]])

            # ---- global routing: masks + per-tile colsums --------------
            m1_all, m2_all = [], []
            with tc.tile_pool(name="ps3", bufs=1, space="PSUM") as PS3:
                cs1_ps = PS3.tile([E, GT * E], f32, tag="cs1")
                cs2_ps = PS3.tile([E, GT * E], f32, tag="cs2")
                for i in range(GT):
                    lg = P_str.tile([128, E], f32, tag="lga")
                    nc.sync.dma_start(lg[:], ag_out[i * 128:(i + 1) * 128, :])
                    rmax = P_str.tile([128, 1], f32, tag="rmax")
                    nc.vector.tensor_reduce(rmax[:], lg[:], AX.X, OP.max)
                    m1 = P_keep.tile([128, E], f32, tag="m1")
                    nc.vector.tensor_scalar(m1[:], lg[:], rmax[:, 0:1], None,
                                            OP.is_equal)
                    m1_all.append(m1)
                    l2n = P_str.tile([128, E], f32, tag="l2n")
                    nc.vector.scalar_tensor_tensor(
                        l2n[:], m1[:], BIG, lg[:], OP.mult, OP.subtract)
                    rmin = P_str.tile([128, 1], f32, tag="rmin")
                    nc.vector.tensor_reduce(rmin[:], l2n[:], AX.X, OP.min)
                    m2 = P_keep.tile([128, E], f32, tag="m2")
                    nc.vector.tensor_scalar(m2[:], l2n[:], rmin[:, 0:1], None,
                                            OP.is_equal)
                    m2_all.append(m2)
                    nc.tensor.matmul(cs1_ps[0:E, i * E:(i + 1) * E],
                                     ones6[:], m1[:], start=True, stop=True)
                    nc.tensor.matmul(cs2_ps[0:E, i * E:(i + 1) * E],
                                     ones6[:], m2[:], start=True, stop=True)
                cs1_sb = P_route.tile([1, GT * E], f32, tag="cs1_sb")
                cs2_sb = P_route.tile([1, GT * E], f32, tag="cs2_sb")
                nc.vector.tensor_copy(cs1_sb[:], cs1_ps[0:1, :])
                nc.vector.tensor_copy(cs2_sb[:], cs2_ps[0:1, :])
            nc.sync.dma_start(cs_dram[0:1, :], cs1_sb[:])
            nc.sync.dma_start(cs_dram[1:2, :], cs2_sb[:])

            # exclusive scan over per-tile colsums + top1 totals
            off_flat1 = P_route.tile([1, GT * E], f32, tag="off_flat1")
            off_flat2 = P_route.tile([1, GT * E], f32, tag="off_flat2")
            tot1_bc = P_route.tile([128, E], f32, tag="tot1bc")
            with tc.tile_pool(name="ps4", bufs=1, space="PSUM") as PS4:
                for s in range(2):
                    colr = P_str.tile([64, E], f32, tag="colr")
                    nc.sync.dma_start(
                        colr[:], cs_dram[s:s + 1, :].rearrange(
                            "p (i e) -> (p i) e", e=E))
                    off_ps = PS4.tile([64, E], f32, tag=f"off{s}", name=f"off{s}")
                    nc.tensor.matmul(off_ps[:], triS_t[:], colr[:],
                                     start=True, stop=True)
                    off_sb = P_str.tile([64, E], f32, tag="off_sb")
                    nc.vector.tensor_copy(off_sb[:], off_ps[:])
                    nc.sync.dma_start(
                        off_dram[s:s + 1, :].rearrange("p (i e) -> (p i) e", e=E),
                        off_sb[:])
                    if s == 0:
                        tot_ps = PS4.tile([E, E], f32, tag="tot")
                        nc.tensor.matmul(tot_ps[:], ones6b[:],
                                         colr[:], start=True, stop=True)
                        tot_sb = P_route.tile([1, E], f32, tag="tot_sb")
                        nc.vector.tensor_copy(tot_sb[:], tot_ps[0:1, :])
                        bc_ps = PS4.tile([128, E], f32, tag="bc")
                        nc.tensor.matmul(bc_ps[:], ones_row[:], tot_sb[:],
                                         start=True, stop=True)
                        nc.vector.tensor_copy(tot1_bc[:], bc_ps[:])
            nc.sync.dma_start(off_flat1[:, :], off_dram[0:1, :])
            nc.sync.dma_start(off_flat2[:, :], off_dram[1:2, :])

            # inclusive cumsums + capacity keep flags -> R_dram [8192, 2]
            with tc.tile_pool(name="ps5", bufs=8, space="PSUM") as PS5:
                for i in range(GT):
                    c1 = PS5.tile([128, E], f32, tag="cum")
                    nc.tensor.matmul(c1[:], tri_t[:], m1_all[i][:],
                                     start=True, stop=False)
                    nc.tensor.matmul(c1[:], ones_row[:],
                                     off_flat1[0:1, i * E:(i + 1) * E],
                                     start=False, stop=True)
                    c2 = PS5.tile([128, E], f32, tag="cum")
                    nc.tensor.matmul(c2[:], tri_t[:], m2_all[i][:],
                                     start=True, stop=False)
                    nc.tensor.matmul(c2[:], ones_row[:],
                                     off_flat2[0:1, i * E:(i + 1) * E],
                                     start=False, stop=True)
                    Rt = P_str.tile([128, 2], f32, tag="Rt")
                    scr = P_str.tile([128, E], f32, tag="scr")
                    a1 = P_str.tile([128, 1], f32, tag="a1")
                    nc.vector.tensor_mul(scr[:], m1_all[i][:], c1[:])
                    nc.vector.tensor_reduce(a1[:], scr[:], AX.X, OP.add)
                    nc.vector.tensor_scalar(Rt[:, 0:1], a1[:], float(CAP),
                                            None, OP.is_le)
                    c2e = P_str.tile([128, E], f32, tag="c2e")
                    nc.vector.tensor_add(c2e[:], c2[:], tot1_bc[:])
                    a2 = P_str.tile([128, 1], f32, tag="a2")
                    nc.vector.tensor_mul(scr[:], m2_all[i][:], c2e[:])
                    nc.vector.tensor_reduce(a2[:], scr[:], AX.X, OP.add)
                    nc.vector.tensor_scalar(Rt[:, 1:2], a2[:], float(CAP),
                                            None, OP.is_le)
                    nc.sync.dma_start(R_dram[i * 128:(i + 1) * 128, :], Rt[:])

            P_keep_cm.__exit__(None, None, None)
            P_route_cm.__exit__(None, None, None)

            # ---- own-token gates & combine weights w -------------------
            for tt in range(NT):
                oix = P_str.tile([128, 1], i32, tag="oix")
                nc.sync.dma_start(oix[:], oidx_d[tt * 128:(tt + 1) * 128, 0:1])
                keep = P_str.tile([128, 2], f32, tag="keepg")
                nc.gpsimd.indirect_dma_start(
                    out=keep[:], out_offset=None, in_=R_dram[:, :],
                    in_offset=bass.IndirectOffsetOnAxis(ap=oix[:, 0:1], axis=0))
                lg = lg_own[tt]
                rmax = P_str.tile([128, 1], f32, tag="rmax")
                nc.vector.tensor_reduce(rmax[:], lg[:], AX.X, OP.max)
                m1 = P_str.tile([128, E], f32, tag="m1o")
                nc.vector.tensor_scalar(m1[:], lg[:], rmax[:, 0:1], None,
                                        OP.is_equal)
                l2n = P_str.tile([128, E], f32, tag="l2n")
                nc.vector.scalar_tensor_tensor(
                    l2n[:], m1[:], BIG, lg[:], OP.mult, OP.subtract)
                rmin = P_str.tile([128, 1], f32, tag="rmin")
                nc.vector.tensor_reduce(rmin[:], l2n[:], AX.X, OP.min)
                m2 = P_str.tile([128, E], f32, tag="m2o")
                nc.vector.tensor_scalar(m2[:], l2n[:], rmin[:, 0:1], None,
                                        OP.is_equal)
                nmax = P_str.tile([128, 1], f32, tag="nmax")
                nc.vector.tensor_reduce(nmax[:], lg[:], AX.X, OP.max,
                                        negate=True)
                gates = P_str.tile([128, E], f32, tag="gates")
                sume = P_str.tile([128, 1], f32, tag="sume")
                nc.scalar.activation(gates[:], lg[:], AF.Exp,
                                     bias=nmax[:, 0:1])
                nc.vector.tensor_reduce(sume[:], gates[:], AX.X, OP.add)
                rsum = P_str.tile([128, 1], f32, tag="rsum")
                nc.vector.reciprocal(rsum[:], sume[:])
                nc.vector.tensor_scalar(gates[:], gates[:], rsum[:, 0:1],
                                        None, OP.mult)
                scr = P_str.tile([128, E], f32, tag="scr2")
                g1 = P_str.tile([128, 1], f32, tag="g1")
                nc.vector.tensor_mul(scr[:], gates[:], m1[:])
                nc.vector.tensor_reduce(g1[:], scr[:], AX.X, OP.add)
                g2 = P_str.tile([128, 1], f32, tag="g2")
                nc.vector.tensor_mul(scr[:], gates[:], m2[:])
                nc.vector.tensor_reduce(g2[:], scr[:], AX.X, OP.add)
                nc.vector.tensor_mul(g1[:], g1[:], keep[:, 0:1])
                nc.vector.tensor_mul(g2[:], g2[:], keep[:, 1:2])
                den = P_str.tile([128, 1], f32, tag="den")
                nc.vector.tensor_add(den[:], g1[:], g2[:])
                nc.vector.tensor_scalar(den[:], den[:], EPS, None, OP.max)
                rden = P_str.tile([128, 1], f32, tag="rden")
                nc.vector.reciprocal(rden[:], den[:])
                nc.vector.tensor_scalar(g1[:], g1[:], rden[:, 0:1], None, OP.mult)
                nc.vector.tensor_scalar(g2[:], g2[:], rden[:, 0:1], None, OP.mult)
                wt = P_str.tile([128, E], f32, tag="wt")
                nc.vector.tensor_scalar(wt[:], m1[:], g1[:, 0:1], None, OP.mult)
                w2 = P_str.tile([128, E], f32, tag="w2")
                nc.vector.tensor_scalar(w2[:], m2[:], g2[:, 0:1], None, OP.mult)
                nc.vector.tensor_add(wt[:], wt[:], w2[:])
                with tc.tile_pool(name=f"pswt_{tt}", bufs=1, space="PSUM") as PSW, \
                     tc.tile_pool(name=f"padwt_{tt}", bufs=1) as P_pw:
                    padw = P_pw.tile([128, 128], f32, tag="padw")
                    nc.vector.tensor_copy(padw[:], pad_t[:])
                    nc.vector.tensor_copy(padw[:, 0:E], wt[:])
                    wtp = PSW.tile([128, 128], f32, tag="wtp")
                    nc.tensor.transpose(wtp[:], padw[:], ident[:])
                    nc.vector.tensor_copy(w_T[:, tt * 128:(tt + 1) * 128],
                                          wtp[0:E, :])

            # ---- expert stage ------------------------------------------
            P_exw_cm = tc.tile_pool(name="exw", bufs=1)
            P_exw = P_exw_cm.__enter__()
            be_t = P_exw.tile([E, M], f32r, tag="be")
            nc.sync.dma_start(be_t[:], be_d[:, :])
            w_T_r = P_exw.tile([E, TPC], f32r, tag="w_T_r")
            nc.vector.tensor_copy(w_T_r[:], w_T[:])
            w_flat = P_exw.tile([1, E * TPC], f32, tag="w_flat")
            nc.sync.dma_start(w_flat[:], w_T[:])   # 6 partition lines -> 1 row

            for tch in range(2):
                tsl = slice(tch * 512, (tch + 1) * 512)
                # broadcast w columns for this token chunk: [128, 512] per e
                wbc = [P_exw.tile([128, 512], f32, tag=f"wbc_{e}",
                                  name=f"wbc_{e}") for e in range(E)]
                with tc.tile_pool(name=f"ps6_{tch}", bufs=2, space="PSUM") as PS6:
                    for e in range(E):
                        wb_ps = PS6.tile([128, 512], f32, tag="wb")
                        nc.tensor.matmul(
                            wb_ps[:], ones_row[:],
                            w_flat[0:1, e * TPC + tch * 512:
                                   e * TPC + (tch + 1) * 512],
                            start=True, stop=True)
                        nc.vector.tensor_copy(wbc[e][:], wb_ps[:])
                with tc.tile_pool(name=f"ps7_{tch}", bufs=8, space="PSUM") as PS7, \
                     tc.tile_pool(name=f"estr_{tch}", bufs=2) as P_es:
                    for nh in range(2):
                        acc = [PS7.tile([128, 512], f32, tag="acc", name="acc")
                               for _ in range(8)]
                        for e in range(E):
                            for kt in range(16):
                                h1w = P_es.tile([128, 512], f32r, tag="h1w")
                                nc.vector.tensor_mul(h1w[:], h1T[kt][:, tsl],
                                                     wbc[e][:])
                                we = P_es.tile([128, 1024], f32r, tag="wes")
                                nc.sync.dma_start(
                                    we[:], We_d[e, kt * 128:(kt + 1) * 128,
                                                nh * 1024:(nh + 1) * 1024])
                                for j in range(8):
                                    nc.tensor.matmul(
                                        acc[j][:], we[:, j * 128:(j + 1) * 128],
                                        h1w[:], start=(e == 0 and kt == 0),
                                        stop=False)
                        for j in range(8):
                            nc.tensor.matmul(
                                acc[j][:],
                                be_t[:, (nh * 8 + j) * 128:(nh * 8 + j + 1) * 128],
                                w_T_r[:, tsl], start=False, stop=True)
                            yst = P_es.tile([128, 512], f32r, tag="yst")
                            nc.vector.tensor_copy(yst[:], acc[j][:])
                            nc.sync.dma_start(
                                y_dram[(nh * 8 + j) * 128:(nh * 8 + j + 1) * 128,
                                       tsl], yst[:])

            P_exw_cm.__exit__(None, None, None)
            P_own_cm.__exit__(None, None, None)
            P_h1_cm.__exit__(None, None, None)

            # ---- final projection + log_softmax ------------------------
            P_z = ctx.enter_context(tc.tile_pool(name="z", bufs=1))
            z_sb = [P_z.tile([128, NCLS], f32, tag=f"z_{tt}", name=f"z_{tt}")
                    for tt in range(NT)]
            bp_t = P_z.tile([1, NCLS], f32r, tag="bp")
            nc.sync.dma_start(bp_t[:], bp_d[:, :])
            with tc.tile_pool(name="ps8", bufs=8, space="PSUM") as PS8, \
                 tc.tile_pool(name="zstr", bufs=3) as P_zs:
                for cch in range(2):
                    c0 = cch * 512
                    wc = min(512, NCLS - c0)
                    acc = [PS8.tile([128, 512], f32, tag="acc", name="acc")
                           for _ in range(NT)]
                    for kt in range(16):
                        wp = P_zs.tile([128, 512], f32r, tag="wps")
                        nc.sync.dma_start(
                            wp[:, 0:wc], Wp_d[kt * 128:(kt + 1) * 128,
                                              c0:c0 + wc])
                        yt = P_zs.tile([128, TPC], f32r, tag="yts")
                        nc.sync.dma_start(yt[:], y_dram[kt * 128:(kt + 1) * 128, :])
                        for tt in range(NT):
                            nc.tensor.matmul(
                                acc[tt][:, 0:wc],
                                yt[:, tt * 128:(tt + 1) * 128],
                                wp[:, 0:wc], start=(kt == 0), stop=False)
                    for tt in range(NT):
                        nc.tensor.matmul(acc[tt][:, 0:wc], ones_row_r[:],
                                         bp_t[0:1, c0:c0 + wc],
                                         start=False, stop=True)
                        nc.vector.tensor_copy(z_sb[tt][:, c0:c0 + wc],
                                              acc[tt][:, 0:wc])

            P_sm = ctx.enter_context(tc.tile_pool(name="smstr", bufs=3))
            for tt in range(NT):
                nmax = P_sm.tile([128, 1], f32, tag="zmax")
                nc.vector.tensor_reduce(nmax[:], z_sb[tt][:], AX.X, OP.max,
                                        negate=True)
                ez = P_sm.tile([128, NCLS], f32, tag="ez")
                sume = P_sm.tile([128, 1], f32, tag="zsum")
                nc.scalar.activation(ez[:], z_sb[tt][:], AF.Exp,
                                     bias=nmax[:, 0:1])
                nc.vector.tensor_reduce(sume[:], ez[:], AX.X, OP.add)
                lns = P_sm.tile([128, 1], f32, tag="lns")
                nc.scalar.activation(lns[:], sume[:], AF.Ln)
                o_t = P_sm.tile([128, NCLS], f32, tag="o_t")
                nc.vector.tensor_scalar(o_t[:], z_sb[tt][:], nmax[:, 0:1],
                                        None, OP.add)
                nc.vector.tensor_scalar(o_t[:], o_t[:], lns[:, 0:1],
                                        None, OP.subtract)
                nc.sync.dma_start(out_d[tt * 128:(tt + 1) * 128, :], o_t[:])

    nc.compile()
    return nc


_CACHE = {}


def _get_nc():
    if "nc" not in _CACHE:
        _CACHE["nc"] = build()
    return _CACHE["nc"]


def prepare_in_maps(x, W0, b0, W1, b1, Wg, We, be, Wp, bp):
    X = np.ascontiguousarray(np.asarray(x, np.float32).reshape(B, IN_DIM))
    shared = dict(
        W0=rne12(W0), b0=np.asarray(b0, np.float32).reshape(M, 1),
        W1=rne12(W1), b1=np.asarray(b1, np.float32).reshape(M, 1),
        Wg=np.asarray(Wg, np.float32),
        We=rne12(We), be=rne12(np.asarray(be, np.float32)),
        Wp=rne12(Wp), bp=rne12(np.asarray(bp, np.float32).reshape(1, NCLS)),
        tri=np.triu(np.ones((128, 128), np.float32)),
        selm=np.repeat(np.eye(E, dtype=np.float32), 128, axis=1),
        triS=np.triu(np.ones((64, 64), np.float32), 1),
    )
    in_maps = []
    for c in range(NCORE):
        xs = X[c * TPC:(c + 1) * TPC]
        in_maps.append(dict(
            shared,
            xT=rne12(np.ascontiguousarray(xs.T)),
            oidx=(c * TPC + np.arange(TPC, dtype=np.int32)).reshape(TPC, 1),
        ))
    return in_maps


def run_cores(inputs, trace=False):
    nc = _get_nc()
    in_maps = prepare_in_maps(**inputs)
    res = bass_utils.run_bass_kernel_spmd(
        nc, in_maps, core_ids=list(range(NCORE)), trace=trace)
    out = np.concatenate([res.results[c]["out"] for c in range(NCORE)], axis=0)
    return out, res


def kernel(**inputs) -> np.ndarray:
    out, _ = run_cores(inputs, trace=False)
    return out

